# revision 4
# baseline (speedup 1.0000x reference)
"""Trainium2 Bass kernel for nn_ClassEmbedding: embedding gather + tanh
feeding a 2-layer LSTM (hidden 512, T=8) over a fused batch of 12800,
data-parallel over 8 NeuronCores (1600 rows/core).

Layout: everything transposed. Gates are computed as
    gatesT[4R, B] = W_ihT-contract(xT) + W_hhT-contract(hT)
so hidden states live as hT/cT [512 -> 4x128 chunks, B] and the recurrence
needs zero transposes. Only the 300-dim embeddings are transposed (PE
transpose, 128-token tiles) with tanh fused into the PSUM->SBUF move.
"""
import sys

sys.path.insert(0, "/opt/trn_rl_repo")

import numpy as np

from concourse import bass, mybir
import concourse.tile as tile
from concourse.bass_utils import run_bass_kernel_spmd
from concourse.masks import make_identity
from concourse.vector_clock import ScopedClock

F32 = mybir.dt.float32
I32 = mybir.dt.int32
AFT = mybir.ActivationFunctionType

P = 128
VOCAB, EMB, RNN, T = 20000, 300, 512, 8
B, NCLS = 64, 200
BN = B * NCLS            # 12800
NCORES = 8
BNC = BN // NCORES       # 1600 per core
PW = 320                 # pass width (batch columns per pass)
NPASS = BNC // PW        # 5
NM = 16                  # 2048 / 128 gate row chunks
EK = [(0, 128), (128, 128), (256, 44)]     # K-chunks of EMB=300
TOKT = [(0, 128), (128, 128), (256, 64)]   # token tiles within a pass

GATE_BUFS = 14
TMP_BUFS = 3
EST_BUFS = 3


def _patched_drain_and_barrier(self, tick_clock, wait_clock):
    # walrus rejects >2 sync waits on one instruction; spread the final
    # drain's waits across single-wait NOPs.
    nc = self.nc
    drain_inst = nc.sync.drain()
    wait_clock.add_sem_waits(
        drain_inst.ins, ScopedClock({None: tick_clock.global_clock})
    )
    si = drain_inst.ins.sync_info
    if si is not None and si.on_wait and len(si.on_wait) > 1:
        waits = list(si.on_wait)
        si.on_wait = waits[:1]
        for w in waits[1:]:
            nop = nc.sync.nop()
            nop.ins.sync_info = mybir.SyncInfo(on_wait=[w], on_update=[])
    nc.all_engine_barrier()
    assert self.sems is not None
    popped = nc._tile_sem_poison_stack.pop()
    assert popped is self._sem_poison
    nc.clear_and_free_semaphores(list(self.sems.allocated().values()))
    nc.all_engine_barrier()


tile.TileContext._drain_and_barrier = _patched_drain_and_barrier


def _split_waits(nc, maxw=1):
    """walrus rejects instructions carrying more than a couple of sync
    waits; keep at most `maxw` on each instruction and move the rest to
    preceding same-engine NOPs."""
    wid = 0
    for bb in nc.main_func.blocks:
        out = []
        changed = False
        for inst in bb.instructions:
            si = inst.sync_info
            if si is not None and si.on_wait and len(si.on_wait) > maxw:
                waits = list(si.on_wait)
                for w in waits[maxw:]:
                    nop = mybir.InstNoOp(name=f"wsplit-{wid}", ins=[], outs=[])
                    wid += 1
                    nop.engine = inst.engine
                    nop.sync_info = mybir.SyncInfo(on_wait=[w], on_update=[])
                    out.append(nop)
                inst.sync_info = mybir.SyncInfo(
                    on_wait=waits[:maxw], on_update=list(si.on_update or [])
                )
                changed = True
            out.append(inst)
        if changed:
            bb.instructions = out


def build_nc():
    nc = bass.Bass()
    w2v = nc.declare_dram_parameter("w2v", [VOCAB, EMB], F32, isOutput=False)
    wih1t = nc.declare_dram_parameter("wih1t", [EMB, 4 * RNN], F32, isOutput=False)
    whh1t = nc.declare_dram_parameter("whh1t", [RNN, 4 * RNN], F32, isOutput=False)
    wih2t = nc.declare_dram_parameter("wih2t", [RNN, 4 * RNN], F32, isOutput=False)
    whh2t = nc.declare_dram_parameter("whh2t", [RNN, 4 * RNN], F32, isOutput=False)
    b1d = nc.declare_dram_parameter("b1", [P, NM], F32, isOutput=False)
    b2d = nc.declare_dram_parameter("b2", [P, NM], F32, isOutput=False)
    idsd = nc.declare_dram_parameter("ids", [P, P], I32, isOutput=False)
    outd = nc.declare_dram_parameter("out", [RNN, BNC], F32, isOutput=True)

    with tile.TileContext(nc) as tc:
        with (
            tc.tile_pool(name="wp", bufs=1) as wp,
            tc.tile_pool(name="sp", bufs=1) as sp,
            tc.tile_pool(name="xp", bufs=2) as xp,
            tc.tile_pool(name="ep", bufs=EST_BUFS) as ep,
            tc.tile_pool(name="gb", bufs=GATE_BUFS) as gb,
            tc.tile_pool(name="tp", bufs=TMP_BUFS) as tp,
            tc.tile_pool(name="gp", bufs=4, space="PSUM") as gp,
            tc.tile_pool(name="tsp", bufs=3, space="PSUM") as tsp,
        ):
            # ---- constants / weights into SBUF ----
            w11 = []
            for i, (ko, kw) in enumerate(EK):
                wt = wp.tile([kw, 4 * RNN], F32, name=f"w11_{i}")
                nc.sync.dma_start(out=wt[:], in_=wih1t[ko : ko + kw, :])
                w11.append(wt)

            def load_rnn_w(dram, label):
                chunks = []
                for i in range(4):
                    wt = wp.tile([P, 4 * RNN], F32, name=f"{label}_{i}")
                    nc.sync.dma_start(out=wt[:], in_=dram[i * P : (i + 1) * P, :])
                    chunks.append(wt)
                return chunks

            w1h = load_rnn_w(whh1t, "w1h")
            w2i = load_rnn_w(wih2t, "w2i")
            w2h = load_rnn_w(whh2t, "w2h")

            b1_sb = wp.tile([P, NM], F32, name="b1_sb")
            nc.sync.dma_start(out=b1_sb[:], in_=b1d[:])
            b2_sb = wp.tile([P, NM], F32, name="b2_sb")
            nc.sync.dma_start(out=b2_sb[:], in_=b2d[:])
            ids_sb = wp.tile([P, P], I32, name="ids_sb")
            nc.sync.dma_start(out=ids_sb[:], in_=idsd[:])
            ident = wp.tile([P, P], F32, name="ident")
            make_identity(nc, ident[:])

            # ---- persistent state tiles ----
            h1 = [
                [sp.tile([P, PW], F32, name=f"h1_{bb}_{r}") for r in range(4)]
                for bb in range(2)
            ]
            h2 = [
                [sp.tile([P, PW], F32, name=f"h2_{bb}_{r}") for r in range(4)]
                for bb in range(2)
            ]
            c1 = [sp.tile([P, PW], F32, name=f"c1_{r}") for r in range(4)]
            c2 = [sp.tile([P, PW], F32, name=f"c2_{r}") for r in range(4)]

            def gen_x(p_, t):
                """Gather 320 token embeddings, transpose to [EMB, PW], tanh."""
                xt = [xp.tile([P, PW], F32, name=f"xt{c}") for c in range(3)]
                for j, (to, tn) in enumerate(TOKT):
                    g = (p_ * T + t) * 3 + j
                    est = ep.tile([P, EMB], F32, name="est")
                    nc.gpsimd.indirect_dma_start(
                        out=est[:tn, :],
                        out_offset=None,
                        in_=w2v[:],
                        in_offset=bass.IndirectOffsetOnAxis(
                            ap=ids_sb[:tn, g : g + 1], axis=0
                        ),
                    )
                    for c, (ko, kw) in enumerate(EK):
                        tpp = tsp.tile([P, P], F32, name="tpp")
                        nc.tensor.transpose(
                            out=tpp[:kw, :tn],
                            in_=est[:tn, ko : ko + kw],
                            identity=ident[:tn, :tn],
                        )
                        nc.scalar.activation(
                            out=xt[c][:kw, to : to + tn],
                            in_=tpp[:kw, :tn],
                            func=AFT.Tanh,
                        )
                return xt

            def do_layer(ks, b_sb, t0flag):
                """ks: list of (lhsT_tile, kw, rhs_tile) accumulated in order.
                Returns dict m -> gate tile [128, PW] (activated, bias added)."""
                ga = {}
                for r in range(4):
                    ms = [r, 8 + r, 12 + r] if t0flag else [r, 4 + r, 8 + r, 12 + r]
                    for mi in ms:
                        ps = gp.tile([P, PW], F32, name="ps")
                        nk = len(ks)
                        for kidx, (lt, kw, rt) in enumerate(ks):
                            nc.tensor.matmul(
                                ps[:],
                                lhsT=lt[:kw, mi * P : (mi + 1) * P],
                                rhs=rt[:kw, :],
                                start=(kidx == 0),
                                stop=(kidx == nk - 1),
                            )
                        func = AFT.Tanh if mi // 4 == 2 else AFT.Sigmoid
                        g = gb.tile([P, PW], F32, name="gt")
                        nc.scalar.activation(
                            out=g[:], in_=ps[:], func=func, bias=b_sb[:, mi : mi + 1]
                        )
                        ga[mi] = g
                return ga

            def update(ga, c, h_new, t0flag):
                for r in range(4):
                    i_, g_, o_ = ga[r], ga[8 + r], ga[12 + r]
                    if t0flag:
                        nc.vector.tensor_mul(out=c[r][:], in0=i_[:], in1=g_[:])
                    else:
                        f_ = ga[4 + r]
                        p1 = tp.tile([P, PW], F32, name="p1")
                        nc.vector.tensor_mul(out=p1[:], in0=f_[:], in1=c[r][:])
                        p2 = tp.tile([P, PW], F32, name="p2")
                        nc.vector.tensor_mul(out=p2[:], in0=i_[:], in1=g_[:])
                        nc.vector.tensor_add(out=c[r][:], in0=p1[:], in1=p2[:])
                    th = tp.tile([P, PW], F32, name="th")
                    nc.scalar.activation(out=th[:], in_=c[r][:], func=AFT.Tanh)
                    nc.vector.tensor_mul(out=h_new[r][:], in0=o_[:], in1=th[:])

            x_cur = gen_x(0, 0)
            for p_ in range(NPASS):
                for t in range(T):
                    wb = t % 2
                    rb = (t - 1) % 2
                    t0 = t == 0
                    # layer 1: x-part first, then hidden part
                    ks1 = [(w11[c], kw, x_cur[c]) for c, (_, kw) in enumerate(EK)]
                    if not t0:
                        ks1 += [(w1h[k], P, h1[rb][k]) for k in range(4)]
                    g1 = do_layer(ks1, b1_sb, t0)
                    update(g1, c1, h1[wb], t0)
                    # prefetch next timestep's x (PE transposes fill the gap
                    # between layer-1 and layer-2 matmuls)
                    if not (p_ == NPASS - 1 and t == T - 1):
                        nt = t + 1
                        npp = p_
                        if nt == T:
                            nt = 0
                            npp = p_ + 1
                        x_next = gen_x(npp, nt)
                    else:
                        x_next = None
                    # layer 2: old-h2 hidden part first, h1-input last
                    ks2 = []
                    if not t0:
                        ks2 += [(w2h[k], P, h2[rb][k]) for k in range(4)]
                    ks2 += [(w2i[k], P, h1[wb][k]) for k in range(4)]
                    g2 = do_layer(ks2, b2_sb, t0)
                    update(g2, c2, h2[wb], t0)
                    x_cur = x_next
                # write this pass's final h2 (t=7 wrote parity 1)
                for r in range(4):
                    nc.sync.dma_start(
                        out=outd[r * P : (r + 1) * P, p_ * PW : (p_ + 1) * PW],
                        in_=h2[1][r][:],
                    )
    _split_waits(nc)
    return nc


_NC_CACHE = None


def _get_nc():
    global _NC_CACHE
    if _NC_CACHE is None:
        _NC_CACHE = build_nc()
    return _NC_CACHE


def _prep_core_inputs(sentence, word2vec, W_ih1, W_hh1, b_ih1, b_hh1,
                      W_ih2, W_hh2, b_ih2, b_hh2):
    f = lambda a: np.ascontiguousarray(np.asarray(a), dtype=np.float32)
    ids_all = np.asarray(sentence).reshape(BN, T).astype(np.int32)
    w2v = f(word2vec)
    wih1t = f(np.asarray(W_ih1, dtype=np.float32).T)
    whh1t = f(np.asarray(W_hh1, dtype=np.float32).T)
    wih2t = f(np.asarray(W_ih2, dtype=np.float32).T)
    whh2t = f(np.asarray(W_hh2, dtype=np.float32).T)
    b1 = f((np.asarray(b_ih1, dtype=np.float32) + np.asarray(b_hh1, dtype=np.float32)).reshape(NM, P).T)
    b2 = f((np.asarray(b_ih2, dtype=np.float32) + np.asarray(b_hh2, dtype=np.float32)).reshape(NM, P).T)

    in_maps = []
    for k in range(NCORES):
        ids_k = ids_all[k * BNC : (k + 1) * BNC]
        ids_arr = np.zeros((P, P), dtype=np.int32)
        for p_ in range(NPASS):
            for t in range(T):
                for j, (to, tn) in enumerate(TOKT):
                    g = (p_ * T + t) * 3 + j
                    ids_arr[:tn, g] = ids_k[p_ * PW + to : p_ * PW + to + tn, t]
        in_maps.append(
            {
                "w2v": w2v,
                "wih1t": wih1t,
                "whh1t": whh1t,
                "wih2t": wih2t,
                "whh2t": whh2t,
                "b1": b1,
                "b2": b2,
                "ids": ids_arr,
            }
        )
    return in_maps


def kernel(sentence, word2vec, W_ih1, W_hh1, b_ih1, b_hh1,
           W_ih2, W_hh2, b_ih2, b_hh2, _trace=False, _return_perf=None):
    nc = _get_nc()
    in_maps = _prep_core_inputs(
        sentence, word2vec, W_ih1, W_hh1, b_ih1, b_hh1, W_ih2, W_hh2, b_ih2, b_hh2
    )
    res = run_bass_kernel_spmd(
        nc, in_maps, core_ids=list(range(NCORES)), trace=_trace
    )
    if _return_perf is not None:
        _return_perf.append(res)
    parts = [res.results[k]["out"].T for k in range(NCORES)]
    out = np.concatenate(parts, axis=0).reshape(B, NCLS, RNN)
    return np.ascontiguousarray(out, dtype=np.float32)


# revision 12
# speedup vs baseline: 3.0456x; 3.0456x over previous
"""Trainium2 Bass kernel for nn_ClassEmbedding: embedding gather + tanh
feeding a 2-layer LSTM (hidden 512, T=8) over a fused batch of 12800,
data-parallel over 8 NeuronCores (1600 rows/core).

Layout: everything transposed. Gates are computed as
    gatesT[4R, B] = W_ihT-contract(xT) + W_hhT-contract(hT)
so hidden states live as hT/cT [512 -> 4x128 chunks, B] and the recurrence
needs zero transposes. Only the 300-dim embeddings are transposed (PE
transpose, 128-token tiles) with tanh fused into the PSUM->SBUF move.
"""
import sys

sys.path.insert(0, "/opt/trn_rl_repo")

import numpy as np

from concourse import bass, mybir
import concourse.tile as tile
from concourse.bass_utils import run_bass_kernel_spmd
from concourse.masks import make_identity
from concourse.vector_clock import ScopedClock

F32 = mybir.dt.float32
I32 = mybir.dt.int32
AFT = mybir.ActivationFunctionType

# matmul operand dtype: plain fp32 matmuls on TRN2 run as HI/LO dual passes
# (4x slower than 16-bit). float32r streams single-pass at the 16-bit rate
# with ~1.2e-4 relative error (measured) -- far better than bf16's 2e-3 --
# and its in-memory bits are plain fp32, so no host-side casting.
MMDT = mybir.dt.float32r
MMNP = np.float32

P = 128
VOCAB, EMB, RNN, T = 20000, 300, 512, 8
B, NCLS = 64, 200
BN = B * NCLS            # 12800
NCORES = 8
BNC = BN // NCORES       # 1600 per core
PW = 320                 # pass width (batch columns per pass)
NPASS = BNC // PW        # 5
NM = 16                  # 2048 / 128 gate row chunks
EK = [(0, 128), (128, 128), (256, 44)]     # K-chunks of EMB=300
TOKT = [(0, 128), (128, 128), (256, 64)]   # token tiles within a pass

GATE_BUFS = 14
TMP_BUFS = 3
EST_BUFS = 3


def _patched_drain_and_barrier(self, tick_clock, wait_clock):
    # walrus rejects >2 sync waits on one instruction; spread the final
    # drain's waits across single-wait NOPs.
    nc = self.nc
    drain_inst = nc.sync.drain()
    wait_clock.add_sem_waits(
        drain_inst.ins, ScopedClock({None: tick_clock.global_clock})
    )
    si = drain_inst.ins.sync_info
    if si is not None and si.on_wait and len(si.on_wait) > 1:
        waits = list(si.on_wait)
        si.on_wait = waits[:1]
        for w in waits[1:]:
            nop = nc.sync.nop()
            nop.ins.sync_info = mybir.SyncInfo(on_wait=[w], on_update=[])
    nc.all_engine_barrier()
    assert self.sems is not None
    popped = nc._tile_sem_poison_stack.pop()
    assert popped is self._sem_poison
    nc.clear_and_free_semaphores(list(self.sems.allocated().values()))
    nc.all_engine_barrier()


tile.TileContext._drain_and_barrier = _patched_drain_and_barrier


def _split_waits(nc, maxw=1):
    """walrus rejects instructions carrying more than a couple of sync
    waits; keep at most `maxw` on each instruction and move the rest to
    preceding same-engine NOPs."""
    wid = 0
    for bb in nc.main_func.blocks:
        out = []
        changed = False
        for inst in bb.instructions:
            si = inst.sync_info
            if si is not None and si.on_wait and len(si.on_wait) > maxw:
                waits = list(si.on_wait)
                for w in waits[maxw:]:
                    nop = mybir.InstNoOp(name=f"wsplit-{wid}", ins=[], outs=[])
                    wid += 1
                    nop.engine = inst.engine
                    nop.sync_info = mybir.SyncInfo(on_wait=[w], on_update=[])
                    out.append(nop)
                inst.sync_info = mybir.SyncInfo(
                    on_wait=waits[:maxw], on_update=list(si.on_update or [])
                )
                changed = True
            out.append(inst)
        if changed:
            bb.instructions = out


def build_nc():
    nc = bass.Bass()
    w2v = nc.declare_dram_parameter("w2v", [VOCAB, EMB], F32, isOutput=False)
    wih1t = nc.declare_dram_parameter("wih1t", [EMB, 4 * RNN], MMDT, isOutput=False)
    whh1t = nc.declare_dram_parameter("whh1t", [RNN, 4 * RNN], MMDT, isOutput=False)
    wih2t = nc.declare_dram_parameter("wih2t", [RNN, 4 * RNN], MMDT, isOutput=False)
    whh2t = nc.declare_dram_parameter("whh2t", [RNN, 4 * RNN], MMDT, isOutput=False)
    b1d = nc.declare_dram_parameter("b1", [P, NM], F32, isOutput=False)
    b2d = nc.declare_dram_parameter("b2", [P, NM], F32, isOutput=False)
    idsd = nc.declare_dram_parameter("ids", [P, P], I32, isOutput=False)
    outd = nc.declare_dram_parameter("out", [RNN, BNC], F32, isOutput=True)

    with tile.TileContext(nc) as tc:
        with (
            tc.tile_pool(name="wp", bufs=1) as wp,
            tc.tile_pool(name="sp", bufs=1) as sp,
            tc.tile_pool(name="xp", bufs=2) as xp,
            tc.tile_pool(name="ep", bufs=EST_BUFS) as ep,
            tc.tile_pool(name="gb", bufs=GATE_BUFS) as gb,
            tc.tile_pool(name="tp", bufs=TMP_BUFS) as tp,
            tc.tile_pool(name="gp", bufs=4, space="PSUM") as gp,
            tc.tile_pool(name="tsp", bufs=3, space="PSUM") as tsp,
        ):
            # ---- constants / weights into SBUF ----
            w11 = []
            for i, (ko, kw) in enumerate(EK):
                wt = wp.tile([kw, 4 * RNN], MMDT, name=f"w11_{i}")
                nc.sync.dma_start(out=wt[:], in_=wih1t[ko : ko + kw, :])
                w11.append(wt)

            def load_rnn_w(dram, label):
                chunks = []
                for i in range(4):
                    wt = wp.tile([P, 4 * RNN], MMDT, name=f"{label}_{i}")
                    nc.sync.dma_start(out=wt[:], in_=dram[i * P : (i + 1) * P, :])
                    chunks.append(wt)
                return chunks

            w1h = load_rnn_w(whh1t, "w1h")
            w2i = load_rnn_w(wih2t, "w2i")
            w2h = load_rnn_w(whh2t, "w2h")

            b1_sb = wp.tile([P, NM], F32, name="b1_sb")
            nc.sync.dma_start(out=b1_sb[:], in_=b1d[:])
            b2_sb = wp.tile([P, NM], F32, name="b2_sb")
            nc.sync.dma_start(out=b2_sb[:], in_=b2d[:])
            ids_sb = wp.tile([P, P], I32, name="ids_sb")
            nc.sync.dma_start(out=ids_sb[:], in_=idsd[:])
            ident32 = wp.tile([P, P], F32, name="ident32")
            make_identity(nc, ident32[:])
            ident = wp.tile([P, P], MMDT, name="ident")
            nc.vector.tensor_copy(out=ident[:], in_=ident32[:])

            # ---- persistent state tiles ----
            h1 = [
                [sp.tile([P, PW], MMDT, name=f"h1_{bb}_{r}") for r in range(4)]
                for bb in range(2)
            ]
            h2 = [
                [sp.tile([P, PW], MMDT, name=f"h2_{bb}_{r}") for r in range(4)]
                for bb in range(2)
            ]
            h2f = [sp.tile([P, PW], F32, name=f"h2f_{r}") for r in range(4)]
            c1 = [sp.tile([P, PW], F32, name=f"c1_{r}") for r in range(4)]
            c2 = [sp.tile([P, PW], F32, name=f"c2_{r}") for r in range(4)]

            def gen_x(p_, t):
                """Gather 320 token embeddings, transpose to [EMB, PW], tanh."""
                xt = [xp.tile([P, PW], MMDT, name=f"xt{c}") for c in range(3)]
                for j, (to, tn) in enumerate(TOKT):
                    g = (p_ * T + t) * 3 + j
                    est = ep.tile([P, EMB], F32, name="est")
                    nc.gpsimd.indirect_dma_start(
                        out=est[:tn, :],
                        out_offset=None,
                        in_=w2v[:],
                        in_offset=bass.IndirectOffsetOnAxis(
                            ap=ids_sb[:tn, g : g + 1], axis=0
                        ),
                    )
                    est2 = ep.tile([P, EMB], MMDT, name="est2")
                    nc.vector.tensor_copy(out=est2[:tn, :], in_=est[:tn, :])
                    for c, (ko, kw) in enumerate(EK):
                        tpp = tsp.tile([P, P], MMDT, name="tpp")
                        nc.tensor.transpose(
                            out=tpp[:kw, :tn],
                            in_=est2[:tn, ko : ko + kw],
                            identity=ident[:tn, :tn],
                        )
                        nc.scalar.activation(
                            out=xt[c][:kw, to : to + tn],
                            in_=tpp[:kw, :tn],
                            func=AFT.Tanh,
                        )
                return xt

            def do_layer(ks, b_sb, t0flag):
                """ks: list of (lhsT_tile, kw, rhs_tile) accumulated in order.
                Returns dict m -> gate tile [128, PW] (activated, bias added)."""
                ga = {}
                for r in range(4):
                    ms = [r, 8 + r, 12 + r] if t0flag else [r, 4 + r, 8 + r, 12 + r]
                    for mi in ms:
                        ps = gp.tile([P, PW], F32, name="ps")
                        nk = len(ks)
                        for kidx, (lt, kw, rt) in enumerate(ks):
                            nc.tensor.matmul(
                                ps[:],
                                lhsT=lt[:kw, mi * P : (mi + 1) * P],
                                rhs=rt[:kw, :],
                                start=(kidx == 0),
                                stop=(kidx == nk - 1),
                            )
                        func = AFT.Tanh if mi // 4 == 2 else AFT.Sigmoid
                        g = gb.tile([P, PW], F32, name="gt")
                        nc.scalar.activation(
                            out=g[:], in_=ps[:], func=func, bias=b_sb[:, mi : mi + 1]
                        )
                        ga[mi] = g
                return ga

            def update(ga, c, h_new, t0flag):
                for r in range(4):
                    i_, g_, o_ = ga[r], ga[8 + r], ga[12 + r]
                    if t0flag:
                        nc.vector.tensor_mul(out=c[r][:], in0=i_[:], in1=g_[:])
                    else:
                        f_ = ga[4 + r]
                        p1 = tp.tile([P, PW], F32, name="p1")
                        nc.vector.tensor_mul(out=p1[:], in0=f_[:], in1=c[r][:])
                        p2 = tp.tile([P, PW], F32, name="p2")
                        nc.vector.tensor_mul(out=p2[:], in0=i_[:], in1=g_[:])
                        nc.vector.tensor_add(out=c[r][:], in0=p1[:], in1=p2[:])
                    th = tp.tile([P, PW], F32, name="th")
                    nc.scalar.activation(out=th[:], in_=c[r][:], func=AFT.Tanh)
                    nc.vector.tensor_mul(out=h_new[r][:], in0=o_[:], in1=th[:])

            x_cur = gen_x(0, 0)
            for p_ in range(NPASS):
                for t in range(T):
                    wb = t % 2
                    rb = (t - 1) % 2
                    t0 = t == 0
                    # layer 1: x-part first, then hidden part
                    ks1 = [(w11[c], kw, x_cur[c]) for c, (_, kw) in enumerate(EK)]
                    if not t0:
                        ks1 += [(w1h[k], P, h1[rb][k]) for k in range(4)]
                    g1 = do_layer(ks1, b1_sb, t0)
                    update(g1, c1, h1[wb], t0)
                    # prefetch next timestep's x (PE transposes fill the gap
                    # between layer-1 and layer-2 matmuls)
                    if not (p_ == NPASS - 1 and t == T - 1):
                        nt = t + 1
                        npp = p_
                        if nt == T:
                            nt = 0
                            npp = p_ + 1
                        x_next = gen_x(npp, nt)
                    else:
                        x_next = None
                    # layer 2: old-h2 hidden part first, h1-input last
                    ks2 = []
                    if not t0:
                        ks2 += [(w2h[k], P, h2[rb][k]) for k in range(4)]
                    ks2 += [(w2i[k], P, h1[wb][k]) for k in range(4)]
                    g2 = do_layer(ks2, b2_sb, t0)
                    # at the last step, h2 is only needed as fp32 output
                    update(g2, c2, h2f if t == T - 1 else h2[wb], t0)
                    x_cur = x_next
                # write this pass's final h2
                for r in range(4):
                    nc.sync.dma_start(
                        out=outd[r * P : (r + 1) * P, p_ * PW : (p_ + 1) * PW],
                        in_=h2f[r][:],
                    )
    _split_waits(nc)
    return nc


_NC_CACHE = None


def _get_nc():
    global _NC_CACHE
    if _NC_CACHE is None:
        _NC_CACHE = build_nc()
    return _NC_CACHE


def _prep_core_inputs(sentence, word2vec, W_ih1, W_hh1, b_ih1, b_hh1,
                      W_ih2, W_hh2, b_ih2, b_hh2):
    f = lambda a: np.ascontiguousarray(np.asarray(a), dtype=np.float32)
    fw = lambda a: np.ascontiguousarray(np.asarray(a, dtype=np.float32).T)
    ids_all = np.asarray(sentence).reshape(BN, T).astype(np.int32)
    w2v = f(word2vec)
    wih1t = fw(W_ih1)
    whh1t = fw(W_hh1)
    wih2t = fw(W_ih2)
    whh2t = fw(W_hh2)
    b1 = f((np.asarray(b_ih1, dtype=np.float32) + np.asarray(b_hh1, dtype=np.float32)).reshape(NM, P).T)
    b2 = f((np.asarray(b_ih2, dtype=np.float32) + np.asarray(b_hh2, dtype=np.float32)).reshape(NM, P).T)

    in_maps = []
    for k in range(NCORES):
        ids_k = ids_all[k * BNC : (k + 1) * BNC]
        ids_arr = np.zeros((P, P), dtype=np.int32)
        for p_ in range(NPASS):
            for t in range(T):
                for j, (to, tn) in enumerate(TOKT):
                    g = (p_ * T + t) * 3 + j
                    ids_arr[:tn, g] = ids_k[p_ * PW + to : p_ * PW + to + tn, t]
        in_maps.append(
            {
                "w2v": w2v,
                "wih1t": wih1t,
                "whh1t": whh1t,
                "wih2t": wih2t,
                "whh2t": whh2t,
                "b1": b1,
                "b2": b2,
                "ids": ids_arr,
            }
        )
    return in_maps


def kernel(sentence, word2vec, W_ih1, W_hh1, b_ih1, b_hh1,
           W_ih2, W_hh2, b_ih2, b_hh2, _trace=False, _return_perf=None):
    nc = _get_nc()
    in_maps = _prep_core_inputs(
        sentence, word2vec, W_ih1, W_hh1, b_ih1, b_hh1, W_ih2, W_hh2, b_ih2, b_hh2
    )
    res = run_bass_kernel_spmd(
        nc, in_maps, core_ids=list(range(NCORES)), trace=_trace
    )
    if _return_perf is not None:
        _return_perf.append(res)
    parts = [res.results[k]["out"].T for k in range(NCORES)]
    out = np.concatenate(parts, axis=0).reshape(B, NCLS, RNN)
    return np.ascontiguousarray(out, dtype=np.float32)


# revision 15
# speedup vs baseline: 3.9353x; 1.2921x over previous
"""Trainium2 Bass kernel for nn_ClassEmbedding: embedding gather + tanh
feeding a 2-layer LSTM (hidden 512, T=8) over a fused batch of 12800,
data-parallel over 8 NeuronCores (1600 rows/core).

Layout: everything transposed. Gates are computed as
    gatesT[4R, B] = W_ihT-contract(xT) + W_hhT-contract(hT)
so hidden states live as hT/cT [512 -> 4x128 chunks, B] and the recurrence
needs zero transposes. Only the 300-dim embeddings are transposed (PE
transpose, 128-token tiles) with tanh fused into the PSUM->SBUF move.
"""
import sys

sys.path.insert(0, "/opt/trn_rl_repo")

import numpy as np

from concourse import bass, mybir
import concourse.tile as tile
from concourse.bass_utils import run_bass_kernel_spmd
from concourse.masks import make_identity
from concourse.vector_clock import ScopedClock

F32 = mybir.dt.float32
I32 = mybir.dt.int32
AFT = mybir.ActivationFunctionType

# matmul operand dtype: plain fp32 matmuls on TRN2 run as HI/LO dual passes
# (4x slower than 16-bit), and fp32-family weights cannot use fast-weight-load
# so LDWEIGHTS leaks ~30ns/matmul into the issue rate. fp16 keeps 10 mantissa
# bits (~2.7e-4 per-matmul rel err measured, vs bf16 2e-3) and runs at the
# full 1 col/cycle rate with LDWEIGHTS completely hidden.
MMDT = mybir.dt.float16
MMNP = np.float16

P = 128
VOCAB, EMB, RNN, T = 20000, 300, 512, 8
B, NCLS = 64, 200
BN = B * NCLS            # 12800
NCORES = 8
BNC = BN // NCORES       # 1600 per core
PW = 320                 # pass width (batch columns per pass)
NPASS = BNC // PW        # 5
NM = 16                  # 2048 / 128 gate row chunks
EMBP = 384                                 # EMB zero-padded for K-chunking
EK = [(0, 128), (128, 128), (256, 44)]     # K-chunks of EMB=300 (data widths)
TOKT = [(0, 128), (128, 128), (256, 64)]   # token tiles within a pass

GATE_BUFS = 14
TMP_BUFS = 3
EST_BUFS = 3


def _patched_drain_and_barrier(self, tick_clock, wait_clock):
    # walrus rejects >2 sync waits on one instruction; spread the final
    # drain's waits across single-wait NOPs.
    nc = self.nc
    drain_inst = nc.sync.drain()
    wait_clock.add_sem_waits(
        drain_inst.ins, ScopedClock({None: tick_clock.global_clock})
    )
    si = drain_inst.ins.sync_info
    if si is not None and si.on_wait and len(si.on_wait) > 1:
        waits = list(si.on_wait)
        si.on_wait = waits[:1]
        for w in waits[1:]:
            nop = nc.sync.nop()
            nop.ins.sync_info = mybir.SyncInfo(on_wait=[w], on_update=[])
    nc.all_engine_barrier()
    assert self.sems is not None
    popped = nc._tile_sem_poison_stack.pop()
    assert popped is self._sem_poison
    nc.clear_and_free_semaphores(list(self.sems.allocated().values()))
    nc.all_engine_barrier()


tile.TileContext._drain_and_barrier = _patched_drain_and_barrier


def _split_waits(nc, maxw=1):
    """walrus rejects instructions carrying more than a couple of sync
    waits; keep at most `maxw` on each instruction and move the rest to
    preceding same-engine NOPs."""
    wid = 0
    for bb in nc.main_func.blocks:
        out = []
        changed = False
        for inst in bb.instructions:
            si = inst.sync_info
            if si is not None and si.on_wait and len(si.on_wait) > maxw:
                waits = list(si.on_wait)
                for w in waits[maxw:]:
                    nop = mybir.InstNoOp(name=f"wsplit-{wid}", ins=[], outs=[])
                    wid += 1
                    nop.engine = inst.engine
                    nop.sync_info = mybir.SyncInfo(on_wait=[w], on_update=[])
                    out.append(nop)
                inst.sync_info = mybir.SyncInfo(
                    on_wait=waits[:maxw], on_update=list(si.on_update or [])
                )
                changed = True
            out.append(inst)
        if changed:
            bb.instructions = out


def build_nc():
    nc = bass.Bass()
    w2v = nc.declare_dram_parameter("w2v", [VOCAB, EMB], F32, isOutput=False)
    wih1t = nc.declare_dram_parameter("wih1t", [EMBP, 4 * RNN], MMDT, isOutput=False)
    whh1t = nc.declare_dram_parameter("whh1t", [RNN, 4 * RNN], MMDT, isOutput=False)
    wih2t = nc.declare_dram_parameter("wih2t", [RNN, 4 * RNN], MMDT, isOutput=False)
    whh2t = nc.declare_dram_parameter("whh2t", [RNN, 4 * RNN], MMDT, isOutput=False)
    b1d = nc.declare_dram_parameter("b1", [P, NM], F32, isOutput=False)
    b2d = nc.declare_dram_parameter("b2", [P, NM], F32, isOutput=False)
    idsd = nc.declare_dram_parameter("ids", [P, P], I32, isOutput=False)
    outd = nc.declare_dram_parameter("out", [RNN, BNC], F32, isOutput=True)

    with tile.TileContext(nc) as tc:
        with (
            tc.tile_pool(name="wp", bufs=1) as wp,
            tc.tile_pool(name="sp", bufs=1) as sp,
            tc.tile_pool(name="xp", bufs=2) as xp,
            tc.tile_pool(name="ep", bufs=EST_BUFS) as ep,
            tc.tile_pool(name="gb", bufs=GATE_BUFS) as gb,
            tc.tile_pool(name="tp", bufs=TMP_BUFS) as tp,
            tc.tile_pool(name="gp", bufs=4, space="PSUM") as gp,
            tc.tile_pool(name="tsp", bufs=3, space="PSUM") as tsp,
        ):
            # ---- constants / weights into SBUF ----
            w11 = []
            for i in range(3):
                wt = wp.tile([P, 4 * RNN], MMDT, name=f"w11_{i}")
                nc.sync.dma_start(out=wt[:], in_=wih1t[i * P : (i + 1) * P, :])
                w11.append(wt)

            def load_rnn_w(dram, label):
                chunks = []
                for i in range(4):
                    wt = wp.tile([P, 4 * RNN], MMDT, name=f"{label}_{i}")
                    nc.sync.dma_start(out=wt[:], in_=dram[i * P : (i + 1) * P, :])
                    chunks.append(wt)
                return chunks

            w1h = load_rnn_w(whh1t, "w1h")
            w2i = load_rnn_w(wih2t, "w2i")
            w2h = load_rnn_w(whh2t, "w2h")

            b1_sb = wp.tile([P, NM], F32, name="b1_sb")
            nc.sync.dma_start(out=b1_sb[:], in_=b1d[:])
            b2_sb = wp.tile([P, NM], F32, name="b2_sb")
            nc.sync.dma_start(out=b2_sb[:], in_=b2d[:])
            ids_sb = wp.tile([P, P], I32, name="ids_sb")
            nc.sync.dma_start(out=ids_sb[:], in_=idsd[:])
            ident32 = wp.tile([P, P], F32, name="ident32")
            make_identity(nc, ident32[:])
            ident = wp.tile([P, P], MMDT, name="ident")
            nc.vector.tensor_copy(out=ident[:], in_=ident32[:])

            # ---- persistent state tiles ----
            h1 = [
                [sp.tile([P, PW], MMDT, name=f"h1_{bb}_{r}") for r in range(4)]
                for bb in range(2)
            ]
            h2 = [
                [sp.tile([P, PW], MMDT, name=f"h2_{bb}_{r}") for r in range(4)]
                for bb in range(2)
            ]
            h2f = [sp.tile([P, PW], F32, name=f"h2f_{r}") for r in range(4)]
            c1 = [sp.tile([P, PW], F32, name=f"c1_{r}") for r in range(4)]
            c2 = [sp.tile([P, PW], F32, name=f"c2_{r}") for r in range(4)]

            def gen_x(p_, t):
                """Gather 320 token embeddings, transpose to [EMB, PW], tanh."""
                xt = [xp.tile([P, PW], MMDT, name=f"xt{c}") for c in range(3)]
                # rows 44:128 of the last chunk are zero-padding for the
                # regularized K=128 matmul
                nc.vector.memset(xt[2][:, :], 0.0)
                for j, (to, tn) in enumerate(TOKT):
                    g = (p_ * T + t) * 3 + j
                    est = ep.tile([P, EMB], F32, name="est")
                    nc.gpsimd.indirect_dma_start(
                        out=est[:tn, :],
                        out_offset=None,
                        in_=w2v[:],
                        in_offset=bass.IndirectOffsetOnAxis(
                            ap=ids_sb[:tn, g : g + 1], axis=0
                        ),
                    )
                    est2 = ep.tile([P, EMB], MMDT, name="est2")
                    nc.vector.tensor_copy(out=est2[:tn, :], in_=est[:tn, :])
                    for c, (ko, kw) in enumerate(EK):
                        tpp = tsp.tile([P, P], MMDT, name="tpp")
                        nc.tensor.transpose(
                            out=tpp[:kw, :tn],
                            in_=est2[:tn, ko : ko + kw],
                            identity=ident[:tn, :tn],
                        )
                        nc.scalar.activation(
                            out=xt[c][:kw, to : to + tn],
                            in_=tpp[:kw, :tn],
                            func=AFT.Tanh,
                        )
                return xt

            def do_layer(ks, b_sb, t0flag):
                """ks: list of (lhsT_tile, kw, rhs_tile) accumulated in order.
                Returns dict m -> gate tile [128, PW] (activated, bias added)."""
                ga = {}
                for r in range(4):
                    ms = [r, 8 + r, 12 + r] if t0flag else [r, 4 + r, 8 + r, 12 + r]
                    for mi in ms:
                        ps = gp.tile([P, PW], F32, name="ps")
                        nk = len(ks)
                        for kidx, (lt, kw, rt) in enumerate(ks):
                            nc.tensor.matmul(
                                ps[:],
                                lhsT=lt[:kw, mi * P : (mi + 1) * P],
                                rhs=rt[:kw, :],
                                start=(kidx == 0),
                                stop=(kidx == nk - 1),
                            )
                        func = AFT.Tanh if mi // 4 == 2 else AFT.Sigmoid
                        g = gb.tile([P, PW], F32, name="gt")
                        nc.scalar.activation(
                            out=g[:], in_=ps[:], func=func, bias=b_sb[:, mi : mi + 1]
                        )
                        ga[mi] = g
                return ga

            def update(ga, c, h_new, t0flag):
                for r in range(4):
                    i_, g_, o_ = ga[r], ga[8 + r], ga[12 + r]
                    if t0flag:
                        nc.vector.tensor_mul(out=c[r][:], in0=i_[:], in1=g_[:])
                    else:
                        f_ = ga[4 + r]
                        p1 = tp.tile([P, PW], F32, name="p1")
                        nc.vector.tensor_mul(out=p1[:], in0=f_[:], in1=c[r][:])
                        p2 = tp.tile([P, PW], F32, name="p2")
                        nc.vector.tensor_mul(out=p2[:], in0=i_[:], in1=g_[:])
                        nc.vector.tensor_add(out=c[r][:], in0=p1[:], in1=p2[:])
                    th = tp.tile([P, PW], F32, name="th")
                    nc.scalar.activation(out=th[:], in_=c[r][:], func=AFT.Tanh)
                    nc.vector.tensor_mul(out=h_new[r][:], in0=o_[:], in1=th[:])

            x_cur = gen_x(0, 0)
            for p_ in range(NPASS):
                for t in range(T):
                    wb = t % 2
                    rb = (t - 1) % 2
                    t0 = t == 0
                    # layer 1: x-part first, then hidden part
                    ks1 = [(w11[c], P, x_cur[c]) for c in range(3)]
                    if not t0:
                        ks1 += [(w1h[k], P, h1[rb][k]) for k in range(4)]
                    g1 = do_layer(ks1, b1_sb, t0)
                    update(g1, c1, h1[wb], t0)
                    # prefetch next timestep's x (PE transposes fill the gap
                    # between layer-1 and layer-2 matmuls)
                    if not (p_ == NPASS - 1 and t == T - 1):
                        nt = t + 1
                        npp = p_
                        if nt == T:
                            nt = 0
                            npp = p_ + 1
                        x_next = gen_x(npp, nt)
                    else:
                        x_next = None
                    # layer 2: old-h2 hidden part first, h1-input last
                    ks2 = []
                    if not t0:
                        ks2 += [(w2h[k], P, h2[rb][k]) for k in range(4)]
                    ks2 += [(w2i[k], P, h1[wb][k]) for k in range(4)]
                    g2 = do_layer(ks2, b2_sb, t0)
                    # at the last step, h2 is only needed as fp32 output
                    update(g2, c2, h2f if t == T - 1 else h2[wb], t0)
                    x_cur = x_next
                # write this pass's final h2
                for r in range(4):
                    nc.sync.dma_start(
                        out=outd[r * P : (r + 1) * P, p_ * PW : (p_ + 1) * PW],
                        in_=h2f[r][:],
                    )
    _split_waits(nc)
    return nc


_NC_CACHE = None


def _get_nc():
    global _NC_CACHE
    if _NC_CACHE is None:
        _NC_CACHE = build_nc()
    return _NC_CACHE


def _prep_core_inputs(sentence, word2vec, W_ih1, W_hh1, b_ih1, b_hh1,
                      W_ih2, W_hh2, b_ih2, b_hh2):
    f = lambda a: np.ascontiguousarray(np.asarray(a), dtype=np.float32)
    fw = lambda a: np.ascontiguousarray(np.asarray(a, dtype=np.float32).T.astype(MMNP))
    ids_all = np.asarray(sentence).reshape(BN, T).astype(np.int32)
    w2v = f(word2vec)
    wih1t = np.zeros((EMBP, 4 * RNN), dtype=MMNP)
    wih1t[:EMB] = fw(W_ih1)
    whh1t = fw(W_hh1)
    wih2t = fw(W_ih2)
    whh2t = fw(W_hh2)
    b1 = f((np.asarray(b_ih1, dtype=np.float32) + np.asarray(b_hh1, dtype=np.float32)).reshape(NM, P).T)
    b2 = f((np.asarray(b_ih2, dtype=np.float32) + np.asarray(b_hh2, dtype=np.float32)).reshape(NM, P).T)

    in_maps = []
    for k in range(NCORES):
        ids_k = ids_all[k * BNC : (k + 1) * BNC]
        ids_arr = np.zeros((P, P), dtype=np.int32)
        for p_ in range(NPASS):
            for t in range(T):
                for j, (to, tn) in enumerate(TOKT):
                    g = (p_ * T + t) * 3 + j
                    ids_arr[:tn, g] = ids_k[p_ * PW + to : p_ * PW + to + tn, t]
        in_maps.append(
            {
                "w2v": w2v,
                "wih1t": wih1t,
                "whh1t": whh1t,
                "wih2t": wih2t,
                "whh2t": whh2t,
                "b1": b1,
                "b2": b2,
                "ids": ids_arr,
            }
        )
    return in_maps


def kernel(sentence, word2vec, W_ih1, W_hh1, b_ih1, b_hh1,
           W_ih2, W_hh2, b_ih2, b_hh2, _trace=False, _return_perf=None):
    nc = _get_nc()
    in_maps = _prep_core_inputs(
        sentence, word2vec, W_ih1, W_hh1, b_ih1, b_hh1, W_ih2, W_hh2, b_ih2, b_hh2
    )
    res = run_bass_kernel_spmd(
        nc, in_maps, core_ids=list(range(NCORES)), trace=_trace
    )
    if _return_perf is not None:
        _return_perf.append(res)
    parts = [res.results[k]["out"].T for k in range(NCORES)]
    out = np.concatenate(parts, axis=0).reshape(B, NCLS, RNN)
    return np.ascontiguousarray(out, dtype=np.float32)


# revision 16
# speedup vs baseline: 4.0329x; 1.0248x over previous
"""Trainium2 Bass kernel for nn_ClassEmbedding: embedding gather + tanh
feeding a 2-layer LSTM (hidden 512, T=8) over a fused batch of 12800,
data-parallel over 8 NeuronCores (1600 rows/core).

Layout: everything transposed. Gates are computed as
    gatesT[4R, B] = W_ihT-contract(xT) + W_hhT-contract(hT)
so hidden states live as hT/cT [512 -> 4x128 chunks, B] and the recurrence
needs zero transposes. Only the 300-dim embeddings are transposed (PE
transpose, 128-token tiles) with tanh fused into the PSUM->SBUF move.
"""
import sys

sys.path.insert(0, "/opt/trn_rl_repo")

import numpy as np

from concourse import bass, mybir
import concourse.tile as tile
from concourse.bass_utils import run_bass_kernel_spmd
from concourse.masks import make_identity
from concourse.vector_clock import ScopedClock

F32 = mybir.dt.float32
I32 = mybir.dt.int32
AFT = mybir.ActivationFunctionType

# matmul operand dtype: plain fp32 matmuls on TRN2 run as HI/LO dual passes
# (4x slower than 16-bit), and fp32-family weights cannot use fast-weight-load
# so LDWEIGHTS leaks ~30ns/matmul into the issue rate. fp16 keeps 10 mantissa
# bits (~2.7e-4 per-matmul rel err measured, vs bf16 2e-3) and runs at the
# full 1 col/cycle rate with LDWEIGHTS completely hidden.
MMDT = mybir.dt.float16
MMNP = np.float16

P = 128
VOCAB, EMB, RNN, T = 20000, 300, 512, 8
B, NCLS = 64, 200
BN = B * NCLS            # 12800
NCORES = 8
BNC = BN // NCORES       # 1600 per core
PW = 400                 # pass width (batch columns per pass)
NPASS = BNC // PW        # 4
NM = 16                  # 2048 / 128 gate row chunks
EMBP = 384                                 # EMB zero-padded for K-chunking
EK = [(0, 128), (128, 128), (256, 44)]     # K-chunks of EMB=300 (data widths)
TOKT = [(0, 128), (128, 128), (256, 128), (384, 16)]  # token tiles per pass

GATE_BUFS = 14
TMP_BUFS = 3
EST_BUFS = 3


def _patched_drain_and_barrier(self, tick_clock, wait_clock):
    # walrus rejects >2 sync waits on one instruction; spread the final
    # drain's waits across single-wait NOPs.
    nc = self.nc
    drain_inst = nc.sync.drain()
    wait_clock.add_sem_waits(
        drain_inst.ins, ScopedClock({None: tick_clock.global_clock})
    )
    si = drain_inst.ins.sync_info
    if si is not None and si.on_wait and len(si.on_wait) > 1:
        waits = list(si.on_wait)
        si.on_wait = waits[:1]
        for w in waits[1:]:
            nop = nc.sync.nop()
            nop.ins.sync_info = mybir.SyncInfo(on_wait=[w], on_update=[])
    nc.all_engine_barrier()
    assert self.sems is not None
    popped = nc._tile_sem_poison_stack.pop()
    assert popped is self._sem_poison
    nc.clear_and_free_semaphores(list(self.sems.allocated().values()))
    nc.all_engine_barrier()


tile.TileContext._drain_and_barrier = _patched_drain_and_barrier


def _split_waits(nc, maxw=1):
    """walrus rejects instructions carrying more than a couple of sync
    waits; keep at most `maxw` on each instruction and move the rest to
    preceding same-engine NOPs."""
    wid = 0
    for bb in nc.main_func.blocks:
        out = []
        changed = False
        for inst in bb.instructions:
            si = inst.sync_info
            if si is not None and si.on_wait and len(si.on_wait) > maxw:
                waits = list(si.on_wait)
                for w in waits[maxw:]:
                    nop = mybir.InstNoOp(name=f"wsplit-{wid}", ins=[], outs=[])
                    wid += 1
                    nop.engine = inst.engine
                    nop.sync_info = mybir.SyncInfo(on_wait=[w], on_update=[])
                    out.append(nop)
                inst.sync_info = mybir.SyncInfo(
                    on_wait=waits[:maxw], on_update=list(si.on_update or [])
                )
                changed = True
            out.append(inst)
        if changed:
            bb.instructions = out


def build_nc():
    nc = bass.Bass()
    w2v = nc.declare_dram_parameter("w2v", [VOCAB, EMB], F32, isOutput=False)
    wih1t = nc.declare_dram_parameter("wih1t", [EMBP, 4 * RNN], MMDT, isOutput=False)
    whh1t = nc.declare_dram_parameter("whh1t", [RNN, 4 * RNN], MMDT, isOutput=False)
    wih2t = nc.declare_dram_parameter("wih2t", [RNN, 4 * RNN], MMDT, isOutput=False)
    whh2t = nc.declare_dram_parameter("whh2t", [RNN, 4 * RNN], MMDT, isOutput=False)
    b1d = nc.declare_dram_parameter("b1", [P, NM], F32, isOutput=False)
    b2d = nc.declare_dram_parameter("b2", [P, NM], F32, isOutput=False)
    idsd = nc.declare_dram_parameter("ids", [P, P], I32, isOutput=False)
    outd = nc.declare_dram_parameter("out", [RNN, BNC], F32, isOutput=True)

    with tile.TileContext(nc) as tc:
        with (
            tc.tile_pool(name="wp", bufs=1) as wp,
            tc.tile_pool(name="sp", bufs=1) as sp,
            tc.tile_pool(name="xp", bufs=2) as xp,
            tc.tile_pool(name="ep", bufs=EST_BUFS) as ep,
            tc.tile_pool(name="gb", bufs=GATE_BUFS) as gb,
            tc.tile_pool(name="tp", bufs=TMP_BUFS) as tp,
            tc.tile_pool(name="gp", bufs=6, space="PSUM") as gp,
            tc.tile_pool(name="tsp", bufs=2, space="PSUM") as tsp,
        ):
            # ---- small constants first: the sync DMA queue is FIFO, and
            # the gather pipeline only needs ids ----
            ids_sb = wp.tile([P, P], I32, name="ids_sb")
            nc.sync.dma_start(out=ids_sb[:], in_=idsd[:])
            b1_sb = wp.tile([P, NM], F32, name="b1_sb")
            nc.sync.dma_start(out=b1_sb[:], in_=b1d[:])
            b2_sb = wp.tile([P, NM], F32, name="b2_sb")
            nc.sync.dma_start(out=b2_sb[:], in_=b2d[:])
            ident32 = wp.tile([P, P], F32, name="ident32")
            make_identity(nc, ident32[:])
            ident = wp.tile([P, P], MMDT, name="ident")
            nc.vector.tensor_copy(out=ident[:], in_=ident32[:])

            # ---- weights into SBUF ----
            w11 = []
            for i in range(3):
                wt = wp.tile([P, 4 * RNN], MMDT, name=f"w11_{i}")
                nc.sync.dma_start(out=wt[:], in_=wih1t[i * P : (i + 1) * P, :])
                w11.append(wt)

            def load_rnn_w(dram, label):
                chunks = []
                for i in range(4):
                    wt = wp.tile([P, 4 * RNN], MMDT, name=f"{label}_{i}")
                    nc.sync.dma_start(out=wt[:], in_=dram[i * P : (i + 1) * P, :])
                    chunks.append(wt)
                return chunks

            w1h = load_rnn_w(whh1t, "w1h")
            w2i = load_rnn_w(wih2t, "w2i")
            w2h = load_rnn_w(whh2t, "w2h")

            # ---- persistent state tiles ----
            h1 = [
                [sp.tile([P, PW], MMDT, name=f"h1_{bb}_{r}") for r in range(4)]
                for bb in range(2)
            ]
            h2 = [
                [sp.tile([P, PW], MMDT, name=f"h2_{bb}_{r}") for r in range(4)]
                for bb in range(2)
            ]
            h2f = [sp.tile([P, PW], F32, name=f"h2f_{r}") for r in range(4)]
            c1 = [sp.tile([P, PW], F32, name=f"c1_{r}") for r in range(4)]
            c2 = [sp.tile([P, PW], F32, name=f"c2_{r}") for r in range(4)]

            def gen_x(p_, t):
                """Gather 320 token embeddings, transpose to [EMB, PW], tanh."""
                xt = [xp.tile([P, PW], MMDT, name=f"xt{c}") for c in range(3)]
                # rows 44:128 of the last chunk are zero-padding for the
                # regularized K=128 matmul
                nc.vector.memset(xt[2][:, :], 0.0)
                for j, (to, tn) in enumerate(TOKT):
                    g = (p_ * T + t) * len(TOKT) + j
                    est = ep.tile([P, EMB], F32, name="est")
                    nc.gpsimd.indirect_dma_start(
                        out=est[:tn, :],
                        out_offset=None,
                        in_=w2v[:],
                        in_offset=bass.IndirectOffsetOnAxis(
                            ap=ids_sb[:tn, g : g + 1], axis=0
                        ),
                    )
                    est2 = ep.tile([P, EMB], MMDT, name="est2")
                    nc.vector.tensor_copy(out=est2[:tn, :], in_=est[:tn, :])
                    for c, (ko, kw) in enumerate(EK):
                        tpp = tsp.tile([P, P], MMDT, name="tpp")
                        nc.tensor.transpose(
                            out=tpp[:kw, :tn],
                            in_=est2[:tn, ko : ko + kw],
                            identity=ident[:tn, :tn],
                        )
                        nc.scalar.activation(
                            out=xt[c][:kw, to : to + tn],
                            in_=tpp[:kw, :tn],
                            func=AFT.Tanh,
                        )
                return xt

            def do_layer(ks, b_sb, t0flag):
                """ks: list of (lhsT_tile, kw, rhs_tile) accumulated in order.
                Returns dict m -> gate tile [128, PW] (activated, bias added)."""
                ga = {}
                for r in range(4):
                    ms = [r, 8 + r, 12 + r] if t0flag else [r, 4 + r, 8 + r, 12 + r]
                    for mi in ms:
                        ps = gp.tile([P, PW], F32, name="ps")
                        nk = len(ks)
                        for kidx, (lt, kw, rt) in enumerate(ks):
                            nc.tensor.matmul(
                                ps[:],
                                lhsT=lt[:kw, mi * P : (mi + 1) * P],
                                rhs=rt[:kw, :],
                                start=(kidx == 0),
                                stop=(kidx == nk - 1),
                            )
                        func = AFT.Tanh if mi // 4 == 2 else AFT.Sigmoid
                        g = gb.tile([P, PW], F32, name="gt")
                        nc.scalar.activation(
                            out=g[:], in_=ps[:], func=func, bias=b_sb[:, mi : mi + 1]
                        )
                        ga[mi] = g
                return ga

            def update(ga, c, h_new, t0flag):
                for r in range(4):
                    i_, g_, o_ = ga[r], ga[8 + r], ga[12 + r]
                    if t0flag:
                        nc.vector.tensor_mul(out=c[r][:], in0=i_[:], in1=g_[:])
                    else:
                        f_ = ga[4 + r]
                        p1 = tp.tile([P, PW], F32, name="p1")
                        nc.vector.tensor_mul(out=p1[:], in0=f_[:], in1=c[r][:])
                        p2 = tp.tile([P, PW], F32, name="p2")
                        nc.vector.tensor_mul(out=p2[:], in0=i_[:], in1=g_[:])
                        nc.vector.tensor_add(out=c[r][:], in0=p1[:], in1=p2[:])
                    th = tp.tile([P, PW], F32, name="th")
                    nc.scalar.activation(out=th[:], in_=c[r][:], func=AFT.Tanh)
                    nc.vector.tensor_mul(out=h_new[r][:], in0=o_[:], in1=th[:])

            x_cur = gen_x(0, 0)
            for p_ in range(NPASS):
                for t in range(T):
                    wb = t % 2
                    rb = (t - 1) % 2
                    t0 = t == 0
                    # layer 1: x-part first, then hidden part
                    ks1 = [(w11[c], P, x_cur[c]) for c in range(3)]
                    if not t0:
                        ks1 += [(w1h[k], P, h1[rb][k]) for k in range(4)]
                    g1 = do_layer(ks1, b1_sb, t0)
                    update(g1, c1, h1[wb], t0)
                    # prefetch next timestep's x (PE transposes fill the gap
                    # between layer-1 and layer-2 matmuls)
                    if not (p_ == NPASS - 1 and t == T - 1):
                        nt = t + 1
                        npp = p_
                        if nt == T:
                            nt = 0
                            npp = p_ + 1
                        x_next = gen_x(npp, nt)
                    else:
                        x_next = None
                    # layer 2: old-h2 hidden part first, h1-input last
                    ks2 = []
                    if not t0:
                        ks2 += [(w2h[k], P, h2[rb][k]) for k in range(4)]
                    ks2 += [(w2i[k], P, h1[wb][k]) for k in range(4)]
                    g2 = do_layer(ks2, b2_sb, t0)
                    # at the last step, h2 is only needed as fp32 output
                    update(g2, c2, h2f if t == T - 1 else h2[wb], t0)
                    x_cur = x_next
                # write this pass's final h2
                for r in range(4):
                    nc.sync.dma_start(
                        out=outd[r * P : (r + 1) * P, p_ * PW : (p_ + 1) * PW],
                        in_=h2f[r][:],
                    )
    _split_waits(nc)
    return nc


_NC_CACHE = None


def _get_nc():
    global _NC_CACHE
    if _NC_CACHE is None:
        _NC_CACHE = build_nc()
    return _NC_CACHE


def _prep_core_inputs(sentence, word2vec, W_ih1, W_hh1, b_ih1, b_hh1,
                      W_ih2, W_hh2, b_ih2, b_hh2):
    f = lambda a: np.ascontiguousarray(np.asarray(a), dtype=np.float32)
    fw = lambda a: np.ascontiguousarray(np.asarray(a, dtype=np.float32).T.astype(MMNP))
    ids_all = np.asarray(sentence).reshape(BN, T).astype(np.int32)
    w2v = f(word2vec)
    wih1t = np.zeros((EMBP, 4 * RNN), dtype=MMNP)
    wih1t[:EMB] = fw(W_ih1)
    whh1t = fw(W_hh1)
    wih2t = fw(W_ih2)
    whh2t = fw(W_hh2)
    b1 = f((np.asarray(b_ih1, dtype=np.float32) + np.asarray(b_hh1, dtype=np.float32)).reshape(NM, P).T)
    b2 = f((np.asarray(b_ih2, dtype=np.float32) + np.asarray(b_hh2, dtype=np.float32)).reshape(NM, P).T)

    in_maps = []
    for k in range(NCORES):
        ids_k = ids_all[k * BNC : (k + 1) * BNC]
        ids_arr = np.zeros((P, P), dtype=np.int32)
        for p_ in range(NPASS):
            for t in range(T):
                for j, (to, tn) in enumerate(TOKT):
                    g = (p_ * T + t) * len(TOKT) + j
                    ids_arr[:tn, g] = ids_k[p_ * PW + to : p_ * PW + to + tn, t]
        in_maps.append(
            {
                "w2v": w2v,
                "wih1t": wih1t,
                "whh1t": whh1t,
                "wih2t": wih2t,
                "whh2t": whh2t,
                "b1": b1,
                "b2": b2,
                "ids": ids_arr,
            }
        )
    return in_maps


def kernel(sentence, word2vec, W_ih1, W_hh1, b_ih1, b_hh1,
           W_ih2, W_hh2, b_ih2, b_hh2, _trace=False, _return_perf=None):
    nc = _get_nc()
    in_maps = _prep_core_inputs(
        sentence, word2vec, W_ih1, W_hh1, b_ih1, b_hh1, W_ih2, W_hh2, b_ih2, b_hh2
    )
    res = run_bass_kernel_spmd(
        nc, in_maps, core_ids=list(range(NCORES)), trace=_trace
    )
    if _return_perf is not None:
        _return_perf.append(res)
    parts = [res.results[k]["out"].T for k in range(NCORES)]
    out = np.concatenate(parts, axis=0).reshape(B, NCLS, RNN)
    return np.ascontiguousarray(out, dtype=np.float32)
